# revision 25
# baseline (speedup 1.0000x reference)
"""Block-sparse attention Trainium2 kernel (8 NeuronCores, SPMD).

Sharding: data-parallel over (batch, head-group): core c handles batch b=c//4
and heads [4*(c%4) .. 4*(c%4)+4). Block index lists are replicated (used
host-side to build the static program). Each core returns a partial
[S, E] output (its heads' contribution through Wo); the host sums the 4
partials per batch and adds bo once.

Design (transposed-scores dataflow):
  Host feeds x^T and W_qkv in bf16. QKV projection runs weights-stationary
  (bf16, 1 cyc/row at 512-wide moving) producing q^T,k^T (bf16, q
  pre-scaled by 1/sqrt(D)) and v^T (bf16); V goes to [keys, d] layout via
  PE transposes. Per head pair, block-diagonal stationaries are built by
  SBUF->SBUF DMA:
    kdiag_j = [[kT_A(j), 0], [0, kT_B(j)]]   (dA|dB x keysA|keysB)
    Vdiag_j = [[V_A(j), 0], [0, V_B(j)]]     (keysA|keysB x dA|dB)
  Active (i,j) blocks are bin-packed into chunks of <=8 row-block pieces
  (<=512 cols, one PSUM bank), separately per i-half so phase B targets a
  2-bank out^T accumulator (two passes i<16 / i>=16, freeing PSUM for a
  3-deep scores ring). Per chunk:
    scoresT = kdiag_j^T @ qT[:, i-cols]   -> PSUM [128=keysA|keysB, cols]
    expT    = ACT exp -> SBUF bf16
    denB    = onesdiag^T @ expT           -> per-head key-sums replicated
                                             across that head's 64 partitions
    rec     = reciprocal_approx_fast(denB)  (custom DVE op, fp32)
    att2T   = expT * rec                  (DVE/GPSIMD split 1:3)
    out^T  += Vdiag_j^T @ att2T           -> PSUM otp [128=dA|dB, 1024]
  No attn transposes, no tensor_reduce, no attnT copies. The pair-0
  front-end (scores..mult) is emitted interleaved with the second half of
  the QKV projection so DVE/GPSIMD/ACT overlap the PE-only phase.
  Wo projection in float32r from out^T tiles; y in bf16; bias on host.
"""
import numpy as np

B, S, E, H, D, BS = 2, 2048, 1024, 16, 64, 64
NB = S // BS          # 32
NCORES = 8
HPC = 4               # heads per core

LAST_RESULTS = None   # BassKernelResults of the most recent run (for test.py)


# ---------------------------------------------------------------- host planning

def _plan(block_rows, block_cols):
    """j-major static schedule shared by every head-pair/core.

    For each col-block j: active row-blocks i, split into maximal
    consecutive runs that do not cross multiples of 8 (PSUM bank alignment
    for the out^T accumulator), grouped into chunks of <=8 blocks
    (<=512 cols, one PSUM bank per scores/den tile).

    Returns:
      chunks: list of (j, [(pos, i0, n), ...]) in emission order; pos is the
              64-col block offset inside the chunk tile.
      flags:  dict (j, i0) -> [start, stop] for the otp2 accumulation.
    """
    mask = np.zeros((NB, NB), dtype=bool)
    for r, c in zip(np.asarray(block_rows).tolist(), np.asarray(block_cols).tolist()):
        mask[int(r), int(c)] = True

    pieces = []          # (j, i0, n) in j-major order
    for j in range(NB):
        ilist = np.nonzero(mask[:, j])[0].tolist()
        cur = None
        for i in ilist:
            if cur is not None and i == cur[1] + cur[2] and (i % 8 != 0):
                cur[2] += 1
            else:
                cur = [j, i, 1]
                pieces.append(cur)
    # bin-pack pieces into chunks of <=8 blocks (first-fit decreasing),
    # separately per i-half so each chunk's phase-B hits one 2-bank otp tile.
    def pack(plist):
        bins = []
        for j, i0, n in sorted(plist, key=lambda p: -p[2]):
            for b in bins:
                if b[0] + n <= 8:
                    b[0] += n
                    b[1].append((j, i0, n))
                    break
            else:
                bins.append([n, [(j, i0, n)]])
        for b in bins:
            b[1].sort()
        bins.sort(key=lambda b: b[1][0])
        out = []
        for _, pl in bins:
            group, nb_ = [], 0
            for j, i0, n in pl:
                group.append((nb_, j, i0, n))
                nb_ += n
            out.append(group)
        return out
    chunks_a = pack([p for p in pieces if p[1] < 16])
    chunks_b = pack([p for p in pieces if p[1] >= 16])
    chunks = chunks_a + chunks_b
    npass_a = len(chunks_a)
    # otp2 start/stop: first/last piece per 8-i bank in emission order
    flags = {}
    first_seen, last_seen = {}, {}
    for group in chunks:
        for pos, j, i0, n in group:
            bk = i0 // 8
            assert (i0 + n - 1) // 8 == bk
            if bk not in first_seen:
                first_seen[bk] = (j, i0)
            last_seen[bk] = (j, i0)
            flags[(j, i0)] = [False, False]
    for bk, key in first_seen.items():
        flags[key][0] = True
    for bk, key in last_seen.items():
        flags[key][1] = True
    return dict(chunks=chunks, flags=flags, npass_a=npass_a)


# ---------------------------------------------------------------- bass program

def _build_program(plan, nsplit=44):
    import concourse.bacc as bacc
    import concourse.mybir as mybir
    from concourse.tile import TileContext
    from concourse import masks

    F32 = mybir.dt.float32
    F32R = mybir.dt.float32r
    BF16 = mybir.dt.bfloat16
    AF = mybir.ActivationFunctionType
    ALU = mybir.AluOpType

    nc = bacc.Bacc("TRN2", target_bir_lowering=False, debug=False)

    xT_in = nc.dram_tensor("xT_local", [E, S], BF16, kind="ExternalInput")
    wqkv_in = nc.dram_tensor("w_qkv", [E, 3 * HPC * D], BF16, kind="ExternalInput")
    bqkv_in = nc.dram_tensor("b_qkv", [3 * HPC * D], F32, kind="ExternalInput")
    wo_in = nc.dram_tensor("w_o", [HPC * D, E], F32R, kind="ExternalInput")
    y_out = nc.dram_tensor("y_partial", [S, E], BF16, kind="ExternalOutput")

    NT = 3 * HPC * D // 128      # 6 qkv n-tiles
    KT = E // 128                # 8 contraction tiles
    ST = S // 128                # 16 s tiles
    SC = S // 512                # 4 s-chunks

    chunks, flags = plan['chunks'], plan['flags']
    npass_a = plan['npass_a']

    with TileContext(nc) as tc:
        with tc.tile_pool(name="const", bufs=1) as cpool, \
             tc.tile_pool(name="qk", bufs=1) as qkpool, \
             tc.tile_pool(name="vt", bufs=1) as vtpool, \
             tc.tile_pool(name="diag", bufs=1) as dgpool, \
             tc.tile_pool(name="outsb", bufs=1) as opool, \
             tc.tile_pool(name="wo", bufs=1) as wop:

            idb = cpool.tile([128, 128], BF16)
            masks.make_identity(nc, idb[:])
            bqkv_sb = cpool.tile([128, NT], F32)
            nc.scalar.dma_start(bqkv_sb[:],
                                bqkv_in.ap().rearrange("(t p) -> p t", p=128))
            bsc = cpool.tile([128, NT], F32)
            nc.scalar.mul(bsc[:, 0:2], bqkv_sb[:, 0:2], 0.125)
            nc.scalar.copy(bsc[:, 2:NT], bqkv_sb[:, 2:NT])
            onesdiag = cpool.tile([128, 128], BF16)
            nc.gpsimd.memset(onesdiag[:], 0.0)
            nc.gpsimd.memset(onesdiag[0:64, 0:64], 1.0)
            nc.gpsimd.memset(onesdiag[64:128, 64:128], 1.0)

            wo_sb = [wop.tile([128, E], F32R, name=f"wo{hp}") for hp in range(2)]
            qT = [qkpool.tile([128, S], BF16, name=f"qT{hp}") for hp in range(2)]
            kT = [qkpool.tile([128, S], BF16, name=f"kT{hp}") for hp in range(2)]
            V = [vtpool.tile([128, (NB // 2) * D], BF16, name=f"V{h}")
                 for h in range(HPC)]
            kdiag = [dgpool.tile([128, NB * 128], BF16, name=f"kdiag{hp}")
                     for hp in range(2)]
            Vdiag = [dgpool.tile([128, NB * 128], BF16, name=f"Vdiag{hp}")
                     for hp in range(2)]
            for hp in range(2):
                nc.vector.memset(kdiag[hp][:], 0.0)
                nc.vector.memset(Vdiag[hp][:], 0.0)
            outSB = [opool.tile([128, S], F32R, name=f"outSB{hp}") for hp in range(2)]

            with tc.tile_pool(name="sc_ps", bufs=3, space="PSUM") as sc_ps, \
                 tc.tile_pool(name="dn_ps", bufs=2, space="PSUM") as dn_ps, \
                 tc.tile_pool(name="ex", bufs=6) as expool, \
                 tc.tile_pool(name="rc", bufs=6) as rcpool, \
                 tc.tile_pool(name="at", bufs=nsplit + 8) as atpool:

                at2_saved = {}

                def emit_front(hp, ci):
                    """scores -> exp -> den -> recip -> mult for one chunk."""
                    group = chunks[ci]
                    ncols = sum(n for _, _, _, n in group) * 64
                    spt = sc_ps.tile([128, 512], F32, tag="spt")
                    for gi, (pos, j, i0, n) in enumerate(group):
                        nc.tensor.matmul(
                            spt[:, pos * 64:(pos + n) * 64],
                            kdiag[hp][:, j * 128:(j + 1) * 128],
                            qT[hp][:, i0 * 64:(i0 + n) * 64],
                            start=(gi == 0), stop=(gi == len(group) - 1))
                    ex = expool.tile([128, 512], BF16, tag="ex")
                    nc.scalar.activation(ex[:, 0:ncols], spt[:, 0:ncols], AF.Exp)
                    dnb = dn_ps.tile([128, 512], F32, tag="dnb")
                    nc.tensor.matmul(dnb[:, 0:ncols], onesdiag[:],
                                     ex[:, 0:ncols], start=True, stop=True)
                    rec = rcpool.tile([128, 512], F32, tag="rec")
                    nc.vector.reciprocal_approx_fast(rec[:, 0:ncols],
                                                     dnb[:, 0:ncols])
                    at2 = atpool.tile([128, 512], BF16, tag="at2")
                    if ci % 4 == 0:
                        nc.vector.tensor_tensor(at2[:, 0:ncols], ex[:, 0:ncols],
                                                rec[:, 0:ncols], ALU.mult)
                    else:
                        nc.gpsimd.tensor_tensor(at2[:, 0:ncols], ex[:, 0:ncols],
                                                rec[:, 0:ncols], ALU.mult)
                    return at2

                def emit_phb(hp, ci, at2, otp2, ibase):
                    for pos, j, i0, n in chunks[ci]:
                        st, sp = flags[(j, i0)]
                        o0 = (i0 - ibase) * 64
                        nc.tensor.matmul(
                            otp2[:, o0:o0 + n * 64],
                            Vdiag[hp][:, j * 128:(j + 1) * 128],
                            at2[:, pos * 64:(pos + n) * 64],
                            start=st, stop=sp)

                # ---- QKV (f32-free: bf16 weights-stationary) ----------------
                qkv_scale = [0.125, 0.125, 1.0, 1.0, 1.0, 1.0]
                with tc.tile_pool(name="xin", bufs=1) as xpool, \
                     tc.tile_pool(name="wq", bufs=1) as wpool, \
                     tc.tile_pool(name="qkv_ps", bufs=2, space="PSUM") as qkv_ps, \
                     tc.tile_pool(name="tr_ps", bufs=1, space="PSUM") as tr_ps:
                    vT = [xpool.tile([128, S], BF16, name=f"vT{hp}")
                          for hp in range(2)]
                    qkv_dst = [qT[0], qT[1], kT[0], kT[1], vT[0], vT[1]]
                    wsb = [wpool.tile([128, 3 * HPC * D], BF16, name=f"w{k}")
                           for k in range(KT)]
                    for k in range(KT):
                        eng = nc.scalar if k % 2 else nc.sync
                        eng.dma_start(wsb[k][:],
                                      wqkv_in.ap()[k * 128:(k + 1) * 128, :])
                    xT_v = xT_in.ap().rearrange("(k p) s -> p k s", p=128)
                    xsc = [xpool.tile([128, KT, 512], BF16, name=f"xsc{sc}")
                           for sc in range(SC)]
                    for sc in range(SC):
                        eng = nc.scalar if sc % 2 else nc.sync
                        eng.dma_start(xsc[sc][:],
                                      xT_v[:, :, sc * 512:(sc + 1) * 512])

                    def emit_qkv_block(t, sc):
                        pt = qkv_ps.tile([128, 512], F32, tag="qkvmm")
                        for k in range(KT):
                            nc.tensor.matmul(
                                pt[:],
                                wsb[k][:, t * 128:(t + 1) * 128],
                                xsc[sc][:, k, :],
                                start=(k == 0), stop=(k == KT - 1))
                        nc.scalar.activation(
                            qkv_dst[t][:, sc * 512:(sc + 1) * 512], pt[:],
                            AF.Identity, bias=bsc[:, t:t + 1],
                            scale=qkv_scale[t])

                    def emit_qkv_t(t):
                        for sc in range(SC):
                            emit_qkv_block(t, sc)

                    def emit_kdiag(hp, quarter=None):
                        kd = kdiag[hp][:, :]
                        for (p0, c0) in ((0, 0), (64, 64)):
                            dst = kd[p0:p0 + 64, :].rearrange(
                                "p (j c) -> p j c", c=128)[:, :, c0:c0 + 64]
                            src = kT[hp][p0:p0 + 64, :].rearrange(
                                "p (j c) -> p j c", c=64)
                            if quarter is None:
                                nc.sync.dma_start(dst, src)
                            else:
                                q8 = quarter * 8
                                nc.sync.dma_start(dst[:, q8:q8 + 8, :],
                                                  src[:, q8:q8 + 8, :])

                    def emit_vprep(vp):
                        for c4 in range(0, NB // 2, 4):
                            tp = tr_ps.tile([128, 512], BF16, tag="vtr")
                            for u in range(4):
                                c = c4 + u
                                nc.tensor.transpose(
                                    tp[:, u * 128:(u + 1) * 128],
                                    vT[vp][:, c * 128:(c + 1) * 128], idb[:])
                            for lh in range(2):
                                src = tp[:, 0:512].rearrange(
                                    "p (u x) -> p u x", x=128)[
                                    :, :, lh * 64:(lh + 1) * 64]
                                dst = V[2 * vp + lh][
                                    :, c4 * 64:(c4 + 4) * 64].rearrange(
                                    "p (u d) -> p u d", d=64)
                                if lh == 0:
                                    nc.scalar.copy(dst, src)
                                else:
                                    nc.vector.tensor_copy(dst, src)
                        vd = Vdiag[vp][:, :]
                        for lh in range(2):
                            h = 2 * vp + lh
                            pd, cd = (0, 0) if lh == 0 else (64, 64)
                            for par in range(2):
                                dst = vd[pd:pd + 64, :].rearrange(
                                    "p (c x) -> p c x", x=256)[
                                    :, :,
                                    par * 128 + cd:par * 128 + cd + 64]
                                src = V[h][par * 64:(par + 1) * 64, :].rearrange(
                                    "p (c d) -> p c d", d=64)
                                nc.sync.dma_start(dst, src)

                    # chunk ci eligible once qT s-chunks <= t0_done and
                    # kdiag quarters <= kq_done are written (subtile deps
                    # enforce correctness; this ordering only aids overlap)
                    nfront = min(nsplit, len(chunks))
                    need_q = [max((i0 + n - 1) // 8 for _, _, i0, n in
                                  chunks[ci]) for ci in range(nfront)]
                    need_k = [max(j // 8 for _, j, _, _ in chunks[ci])
                              for ci in range(nfront)]
                    next_ci = 0

                    def emit_eligible(t0_done, kq_done, cap):
                        nonlocal next_ci
                        done = 0
                        while (next_ci < nfront and done < cap
                               and need_q[next_ci] < t0_done * 2
                               and need_k[next_ci] < kq_done):
                            at2_saved[next_ci] = emit_front(0, next_ci)
                            next_ci += 1
                            done += 1

                    emit_qkv_block(0, 0)
                    emit_qkv_block(0, 1)
                    for q in range(SC):
                        emit_qkv_block(2, q)
                        emit_kdiag(0, quarter=q)
                        emit_eligible(1, q + 1, 4)
                    blocks_rest = ([(0, 2), (0, 3)]
                                   + [(t, sc) for t in (4, 1, 3, 5)
                                      for sc in range(SC)])
                    bi_ = 0

                    def drain_block():
                        nonlocal bi_
                        t, sc = blocks_rest[bi_]
                        emit_qkv_block(t, sc)
                        bi_ += 1
                        if (t, sc) == (4, SC - 1):
                            emit_vprep(0)
                        elif (t, sc) == (3, SC - 1):
                            emit_kdiag(1)
                        elif (t, sc) == (5, SC - 1):
                            emit_vprep(1)

                    while next_ci < nfront or bi_ < len(blocks_rest):
                        t0d = 1 + min(bi_, 2)   # (0,2)/(0,3) are first in rest
                        emit_eligible(t0d, 4, 3)
                        if bi_ < len(blocks_rest):
                            drain_block()
                        elif next_ci < nfront:
                            at2_saved[next_ci] = emit_front(0, next_ci)
                            next_ci += 1

                for hp in range(2):
                    eng = nc.scalar if hp else nc.sync
                    eng.dma_start(wo_sb[hp][:],
                                  wo_in.ap()[hp * 128:(hp + 1) * 128, :])

                # ---- attention phase B + pair 1 -----------------------------
                with tc.tile_pool(name="ot_ps", bufs=1, space="PSUM") as ot_ps:
                    for hp in range(2):
                        if hp == 0:
                            for ci in range(nsplit,
                                            min(nsplit + 8, len(chunks))):
                                at2_saved[ci] = emit_front(0, ci)
                        for lo, hi, ibase in ((0, npass_a, 0),
                                              (npass_a, len(chunks), 16)):
                            otp2 = ot_ps.tile([128, S // 2], F32, tag="otp")
                            for ci in range(lo, hi):
                                if hp == 0 and ci in at2_saved:
                                    at2 = at2_saved.pop(ci)
                                else:
                                    at2 = emit_front(hp, ci)
                                emit_phb(hp, ci, at2, otp2, ibase)
                            for q in range(2):
                                dst = outSB[hp][:, ibase * 64 + q * 512:
                                                ibase * 64 + (q + 1) * 512]
                                if q == 0:
                                    nc.scalar.copy(dst, otp2[:, q * 512:(q + 1) * 512])
                                else:
                                    nc.vector.tensor_copy(
                                        dst, otp2[:, q * 512:(q + 1) * 512])

                # ---- Wo projection (f32r), bias added on host ---------------
                with tc.tile_pool(name="yt", bufs=4) as ypool, \
                     tc.tile_pool(name="wo_ps", bufs=3, space="PSUM") as wo_ps:
                    for st_ in range(ST):
                        yt = ypool.tile([128, E], BF16, tag="yt")
                        for nchk in range(2):
                            pt = wo_ps.tile([128, 512], F32, tag="womm")
                            for hp in range(2):
                                nc.tensor.matmul(
                                    pt[:],
                                    outSB[hp][:, st_ * 128:(st_ + 1) * 128],
                                    wo_sb[hp][:, nchk * 512:(nchk + 1) * 512],
                                    start=(hp == 0), stop=(hp == 1))
                            if nchk == 0:
                                nc.scalar.copy(yt[:, nchk * 512:(nchk + 1) * 512],
                                               pt[:])
                            else:
                                nc.vector.tensor_copy(
                                    yt[:, nchk * 512:(nchk + 1) * 512], pt[:])
                        eng = nc.gpsimd if st_ % 2 else nc.sync
                        eng.dma_start(y_out.ap()[st_ * 128:(st_ + 1) * 128, :],
                                      yt[:])

    nc.compile()
    return nc


# ---------------------------------------------------------------- entry point

def kernel(x, Wq, bq, Wk, bk, Wv, bv, Wo, bo, block_rows, block_cols):
    global LAST_RESULTS
    from concourse.bass_utils import run_bass_kernel_spmd
    import os

    x = np.asarray(x, dtype=np.float32)
    Wq, Wk, Wv, Wo = (np.asarray(a, dtype=np.float32) for a in (Wq, Wk, Wv, Wo))
    bq, bk, bv, bo = (np.asarray(a, dtype=np.float32) for a in (bq, bk, bv, bo))

    plan = _plan(block_rows, block_cols)
    nc = _build_program(plan)

    import ml_dtypes
    bf16 = ml_dtypes.bfloat16
    xT = [np.ascontiguousarray(x[b].T).astype(bf16) for b in range(B)]
    in_maps = []
    for c in range(NCORES):
        b, g = c // 4, c % 4
        cs = slice(g * HPC * D, (g + 1) * HPC * D)
        w_qkv = np.ascontiguousarray(
            np.concatenate([Wq[:, cs], Wk[:, cs], Wv[:, cs]], axis=1)).astype(bf16)
        b_qkv = np.ascontiguousarray(
            np.concatenate([bq[cs], bk[cs], bv[cs]]))
        w_o = np.ascontiguousarray(Wo[cs, :])
        in_maps.append(dict(xT_local=xT[b], w_qkv=w_qkv, b_qkv=b_qkv, w_o=w_o))

    trace = bool(int(os.environ.get("KERNEL_TRACE", "0")))
    res = run_bass_kernel_spmd(nc, in_maps, core_ids=list(range(NCORES)),
                               trace=trace)
    LAST_RESULTS = res

    y = np.zeros((B, S, E), dtype=np.float32)
    for c in range(NCORES):
        y[c // 4] += np.asarray(res.results[c]["y_partial"], dtype=np.float32)
    y += bo
    return y
